# revision 78
# baseline (speedup 1.0000x reference)
"""AngleUpdate v2: dma_gather-based GNN message-passing kernel, 8 TRN2 cores.

Data-parallel over angles. Per core, angles are host-permuted into 128
buckets keyed by (bond-window of src pair, bond-window of dst pair,
atom-window of vtx pair); bucket capacities are cross-core maxima (fixed
program, per-core data fills with dummies). Gathers use the SWDGE
dma_gather ucode (non-transpose): int16 window-local pair indices, bf16
pair-packed tables, <=896 rows/instruction, 4 parallel SWDGE queues.

Per superblock (4096 angles, J=32 slots of 128):
  - DMA idx (3x[128,256] i16), masks ([128,3,32] u8), afT ([64,4096] bf16)
  - windowed dma_gathers land pair tiles P_s/P_d/P_v [128,J,128] bf16
  - pair half-select via tensor_copy + copy_predicated (mask free-dim bcast)
    packing xin1=[src|dst], P_v in place; afT DMA'd into xt2[64:128]
  - PE transposes build xt1 [128,4096] (via PSUM) and xt2[0:64]=vtx^T
  - mm1: h = relu(W1^T x + b1) per 512-col subtile (2 K-chunks, 2 MLPs)
  - mm2 feature-major: [64,512] = W2^T h; sigmoid/silu + b2 per partition
  - res = prod + afT; out bf16 feature-major -> host transposes/unpermutes
"""

import sys

sys.path.insert(0, "/opt/trn_rl_repo")

from contextlib import ExitStack

import numpy as np

import concourse.bass as bass
import concourse.mybir as mybir
import concourse.tile as tile
from concourse import bacc

N_CORES = 8
N_ANGLES = 1_000_000
A_CORE = N_ANGLES // N_CORES
N_BONDS = 500_000
N_ATOMS = 100_000
D = 64
HID = 128

PAIR_B = N_BONDS // 2   # 250000 bond pairs
PAIR_A = N_ATOMS // 2   # 50000 atom pairs
WIN = 32768
WB = (PAIR_B + WIN - 1) // WIN  # 8 bond windows
WA = (PAIR_A + WIN - 1) // WIN  # 2 atom windows
NKEY = WB * WB * WA             # 128 buckets

SB = 4096        # superblock angles
J = SB // 128    # 32 slots
SEG_MAX = 896    # dma_gather per-instruction row cap. 1024 is functional but
                 # ~6% slower: it exactly fills the desc ring, so generation
                 # stalls on drain instead of pipelining.
SUB = 512        # compute subtile columns

F32 = mybir.dt.float32
BF16 = mybir.dt.bfloat16
I16 = mybir.dt.int16
U8 = mybir.dt.uint8

Relu = mybir.ActivationFunctionType.Relu
Sigmoid = mybir.ActivationFunctionType.Sigmoid
Silu = mybir.ActivationFunctionType.Silu


# ---------------------------------------------------------------- host layout

def build_layout(src, dst, vtx):
    """src/dst/vtx: [N_ANGLES] int64 global index arrays.
    Angles sharing a bucket key are interchangeable, so they are dealt
    round-robin across cores per bucket: per-core bucket counts become
    floor/ceil(total/8), collapsing the cross-core-max capacity padding
    (~15% -> ~5%). Returns per-core packed device arrays (angle_of_col
    holds GLOBAL angle ids) + fixed segment lists."""
    sp, dp, vp = src >> 1, dst >> 1, vtx >> 1
    ws, wd, wv = sp // WIN, dp // WIN, vp // WIN
    key = ((ws * WB + wd) * WA + wv).astype(np.int64)  # [N]

    g_order = np.argsort(key, kind="stable")
    kcounts = np.bincount(key, minlength=NKEY)
    gstart = np.concatenate([[0], np.cumsum(kcounts)])
    rank = np.arange(len(key)) - gstart[key[g_order]]
    core_of = np.empty(len(key), np.int64)
    core_of[g_order] = rank % N_CORES

    counts = np.zeros((N_CORES, NKEY), np.int64)
    for c in range(N_CORES):
        counts[c] = np.bincount(key[core_of == c], minlength=NKEY)
    maxcnt = counts.max(axis=0)
    cap = ((maxcnt + 127) // 128) * 128
    a_layout = int(cap.sum())
    a_pad = ((a_layout + SB - 1) // SB) * SB
    cap[-1] += a_pad - a_layout  # absorb tail into last bucket
    offs = np.concatenate([[0], np.cumsum(cap)]).astype(np.int64)
    A = a_pad
    # columns in [a_layout, A) are structural padding (dummy on every core,
    # 128-aligned) — never gather them
    # fixed segment lists per stream: (col0, length, window)
    keys = np.arange(NKEY)
    kws, kwd, kwv = keys // (WB * WA), (keys // WA) % WB, keys % WA
    # NOTE: raising any stream's cap to 1024 (to make v-buckets single
    # instructions) measures ~2.6% slower — exact-ring-fill gathers stall
    # descriptor generation on drain. Keep 896 everywhere.
    seg_cap = {"s": SEG_MAX, "d": SEG_MAX, "v": SEG_MAX}
    segs = {"s": [], "d": [], "v": []}
    for name, w_of in (("s", kws), ("d", kwd), ("v", kwv)):
        runs = []  # merge adjacent buckets with same window
        for k in range(NKEY):
            if cap[k] == 0:
                continue
            c0 = int(offs[k])
            c1 = min(int(offs[k + 1]), a_layout)
            w = int(w_of[k])
            if c1 <= c0:
                continue
            if runs and runs[-1][2] == w and runs[-1][1] == c0:
                runs[-1][1] = c1
            else:
                runs.append([c0, c1, w])
        for c0, c1, w in runs:
            x = c0
            while x < c1:
                nxt_sb = (x // SB + 1) * SB
                e = min(c1, nxt_sb, x + seg_cap[name])
                segs[name].append((x, e - x, w))
                x = e

    # per-core packed arrays
    pack = []
    for c in range(N_CORES):
        gids_c = np.flatnonzero(core_of == c)  # global angle ids of this core
        order = np.argsort(key[gids_c], kind="stable")
        kc = counts[c]
        start = 0
        # fill columns bucket by bucket
        loc_s = np.zeros(A, np.int16)
        loc_d = np.zeros(A, np.int16)
        loc_v = np.zeros(A, np.int16)
        m_s = np.zeros(A, np.uint8)
        m_d = np.zeros(A, np.uint8)
        m_v = np.zeros(A, np.uint8)
        angle_of_col = np.full(A, -1, np.int64)
        for k in range(NKEY):
            n = int(kc[k])
            if n == 0:
                continue
            ids = gids_c[order[start : start + n]]  # global ids, bucket k
            start += n
            o = int(offs[k])
            cols = np.arange(o, o + n)
            angle_of_col[cols] = ids
            loc_s[cols] = (sp[ids] - (sp[ids] // WIN) * WIN).astype(np.int16)
            loc_d[cols] = (dp[ids] - (dp[ids] // WIN) * WIN).astype(np.int16)
            loc_v[cols] = (vp[ids] - (vp[ids] // WIN) * WIN).astype(np.int16)
            m_s[cols] = (src[ids] & 1).astype(np.uint8)
            m_d[cols] = (dst[ids] & 1).astype(np.uint8)
            m_v[cols] = (vtx[ids] & 1).astype(np.uint8)

        # mark dummies that are trailing within their gather instruction as -1
        # (the SWDGE ucode trims trailing negatives -> fewer descriptors).
        # Guard: never trim a whole instruction to zero (crashes the ucode).
        import os

        # NOTE: the SWDGE trailing-negative trim path hangs the device
        # (NRT_EXEC_UNIT_UNRECOVERABLE) even with 16-aligned trim counts.
        # Keep TRIM off.
        dm = (angle_of_col < 0) & (os.environ.get("TRIM", "0") == "1")
        for name, loc in (("s", loc_s), ("d", loc_d), ("v", loc_v)):
            for col0, ln, _w in segs[name]:
                e = col0 + ln
                t = 0
                while t < ln and dm[e - 1 - t]:
                    t += 1
                t = min(t, ln - 64)
                t = (t // 16) * 16
                if t > 0:
                    loc[e - t : e] = -1

        def pack16(loc):
            # idx i at [(i%16)+16k, i//16]
            a = loc.reshape(-1, 16).T  # [16, A/16]
            return np.ascontiguousarray(np.tile(a, (8, 1)))

        def pack128(*ms):
            # [128, n_streams, A/128]; m[i] at [i%128, s, i//128]
            return np.ascontiguousarray(
                np.stack([m.reshape(-1, 128).T for m in ms], axis=1)
            )

        pack.append(
            dict(
                idx_s=pack16(loc_s), idx_d=pack16(loc_d), idx_v=pack16(loc_v),
                masks=pack128(m_s, m_d, m_v),
                angle_of_col=angle_of_col,
            )
        )
    return dict(A=A, cap=cap, offs=offs, segs=segs, pack=pack)


def ordered_segs(segs, A):
    """Global device-iteration order of gather segments: per superblock,
    all streams' segments sorted by length desc (stable)."""
    n_sb = A // SB
    seg_by_sb = {s: [[] for _ in range(n_sb)] for s in "sdv"}
    for s in "sdv":
        for col0, ln, w in segs[s]:
            seg_by_sb[s][col0 // SB].append((col0, ln, w))
    out = []
    for sb_i in range(n_sb):
        segs_here = [
            (s, col0, ln, w)
            for s in "sdv"
            for col0, ln, w in seg_by_sb[s][sb_i]
        ]
        segs_here.sort(key=lambda t: -t[2])
        out.append(segs_here)
    return out


def pair_pack(tab):
    """[N, 64] f32 -> [N/2, 128] bf16 pairs."""
    import ml_dtypes

    return np.ascontiguousarray(
        tab.reshape(-1, 128).astype(ml_dtypes.bfloat16)
    )


# ---------------------------------------------------------------- device build

def build_nc(A, segs, debug_taps=False):
    import ml_dtypes  # noqa: F401

    n_sb = A // SB
    nc = bacc.Bacc(
        "TRN2", target_bir_lowering=False, debug=False,
        num_swdge_queues=4,
    )

    bondP = nc.dram_tensor("bondP", [PAIR_B, 128], BF16, kind="ExternalInput").ap()
    atomP = nc.dram_tensor("atomP", [PAIR_A, 128], BF16, kind="ExternalInput").ap()
    afT = nc.dram_tensor("afT", [D, A], BF16, kind="ExternalInput").ap()
    idx_s = nc.dram_tensor("idx_s", [128, A // 16], I16, kind="ExternalInput").ap()
    idx_d = nc.dram_tensor("idx_d", [128, A // 16], I16, kind="ExternalInput").ap()
    idx_v = nc.dram_tensor("idx_v", [128, A // 16], I16, kind="ExternalInput").ap()
    masks = nc.dram_tensor("masks", [128, 3, A // 128], U8, kind="ExternalInput").ap()
    w1c1 = nc.dram_tensor("w1c1", [128, 2, HID], F32, kind="ExternalInput").ap()
    w1c2 = nc.dram_tensor("w1c2", [128, 2, HID], F32, kind="ExternalInput").ap()
    gb1 = nc.dram_tensor("gb1", [HID], F32, kind="ExternalInput").ap()
    ob1 = nc.dram_tensor("ob1", [HID], F32, kind="ExternalInput").ap()
    gW2 = nc.dram_tensor("gW2", [HID, D], F32, kind="ExternalInput").ap()
    oW2 = nc.dram_tensor("oW2", [HID, D], F32, kind="ExternalInput").ap()
    gb2 = nc.dram_tensor("gb2", [D], F32, kind="ExternalInput").ap()
    ob2 = nc.dram_tensor("ob2", [D], F32, kind="ExternalInput").ap()
    ident_d = nc.dram_tensor("ident128", [128, 128], BF16, kind="ExternalInput").ap()
    sb_segs = ordered_segs(segs, A)
    outT = nc.dram_tensor("outT", [D, A], BF16, kind="ExternalOutput").ap()
    dbg = {}
    if debug_taps:
        for nm, shape in [
            ("dbg_xin", [128, 2 * J * 128]), ("dbg_pv", [128, J * 128]),
            ("dbg_xt1", [128, SB]), ("dbg_xt2", [128, SB]),
            ("dbg_gs", [64, SB]), ("dbg_so", [64, SB]), ("dbg_ob", [64, SB]),
        ]:
            dbg[nm] = nc.dram_tensor(nm, shape, BF16, kind="ExternalOutput").ap()

    with tile.TileContext(nc) as tc, ExitStack() as ctx:
        cpool = ctx.enter_context(tc.tile_pool(name="const", bufs=1))

        ident = cpool.tile([128, 128], BF16)
        nc.sync.dma_start(out=ident[:], in_=ident_d)

        w1f = cpool.tile([128, 2, 2, HID], F32)
        nc.sync.dma_start(out=w1f[:, 0, :, :], in_=w1c1)
        nc.sync.dma_start(out=w1f[:, 1, :, :], in_=w1c2)
        w1b = cpool.tile([128, 2, 2, HID], BF16)
        nc.vector.tensor_copy(out=w1b[:], in_=w1f[:])
        # w1b[:, chunk, mlp, :]

        w2f = cpool.tile([128, 2, D], F32)
        nc.sync.dma_start(out=w2f[:, 0, :], in_=gW2)
        nc.sync.dma_start(out=w2f[:, 1, :], in_=oW2)
        w2b = cpool.tile([128, 2, D], BF16)
        nc.scalar.copy(out=w2b[:], in_=w2f[:])
        gw2t, ow2t = w2b[:, 0, :], w2b[:, 1, :]

        gb1t = cpool.tile([128, 1], F32)
        nc.sync.dma_start(out=gb1t[:], in_=gb1.rearrange("(p o) -> p o", o=1))
        ob1t = cpool.tile([128, 1], F32)
        nc.sync.dma_start(out=ob1t[:], in_=ob1.rearrange("(p o) -> p o", o=1))
        gb2t = cpool.tile([64, 1], F32)
        nc.sync.dma_start(out=gb2t[:], in_=gb2.rearrange("(p o) -> p o", o=1))
        ob2t = cpool.tile([64, 1], F32)
        nc.sync.dma_start(out=ob2t[:], in_=ob2.rearrange("(p o) -> p o", o=1))
        # stacked [gb2; ob2] for one 128-partition sigmoid over both MLP halves
        b2t = cpool.tile([128, 1], F32)
        nc.vector.tensor_copy(out=b2t[0:64, :], in_=gb2t[:])
        nc.vector.tensor_copy(out=b2t[64:128, :], in_=ob2t[:])

        idx_pool = ctx.enter_context(tc.tile_pool(name="idx", bufs=4))
        msk_pool = ctx.enter_context(tc.tile_pool(name="msk", bufs=4))
        g_pool = ctx.enter_context(tc.tile_pool(name="gp", bufs=4))
        x_pool = ctx.enter_context(tc.tile_pool(name="xp", bufs=4))
        tp_psum = ctx.enter_context(tc.tile_pool(name="tpp", bufs=2, space="PSUM"))
        h_psum = ctx.enter_context(tc.tile_pool(name="hp", bufs=2, space="PSUM"))
        h_pool = ctx.enter_context(tc.tile_pool(name="hs", bufs=2))
        m2_psum = ctx.enter_context(tc.tile_pool(name="m2p", bufs=2, space="PSUM"))
        ep_pool = ctx.enter_context(tc.tile_pool(name="ep", bufs=2))

        dram_in = {"s": bondP, "d": bondP, "v": atomP}
        idx_in = {"s": idx_s, "d": idx_d, "v": idx_v}
        # LPT queue balancing: descriptor-gen cost ~ rows + ~130 rows worth of
        # fixed overhead per instruction; keep the 4 SWDGE contexts even.
        qload = [0, 0, 0, 0]

        # idx/mask loads are issued PRE superblocks ahead of use so they sit
        # in the sync HWDGE FIFO in front of the epilogue-gated outT stores
        # (head-of-line blocking otherwise collapses prefetch to one SB).
        PRE = 3

        def load_meta(k):
            kb = k * SB
            ti = {}
            for s in "sdv":
                t = idx_pool.tile([128, SB // 16], I16, tag=f"ti{s}")
                nc.sync.dma_start(
                    out=t[:], in_=idx_in[s][:, kb // 16 : (kb + SB) // 16]
                )
                ti[s] = t
            mt = msk_pool.tile([128, 3, J], U8, tag="mk")
            nc.sync.dma_start(
                out=mt[:], in_=masks[:, :, kb // 128 : (kb + SB) // 128]
            )
            return ti, mt

        meta_q = [load_meta(k) for k in range(min(PRE, n_sb))]

        for sb_i in range(n_sb):
            base = sb_i * SB

            if sb_i + PRE < n_sb:
                meta_q.append(load_meta(sb_i + PRE))
            ti, mt = meta_q.pop(0)

            # xin: [partition, stream(src/dst), slot, half(top/bot), feat]
            xin = g_pool.tile([128, 2, J, 2, 64], BF16, tag="xin")
            P_v = g_pool.tile([128, J, 128], BF16, tag="Pv")
            for s, col0, ln, w in sb_segs[sb_i]:
                tab = dram_in[s]
                nrow = tab.shape[0]
                j0 = (col0 - base) // 128
                w0 = w * WIN
                w1 = min(w0 + WIN, nrow)
                q = min(range(4), key=lambda i: qload[i])
                qload[q] += ln + 130
                if s == "v":
                    o = P_v[:, j0 : j0 + ln // 128, :]
                else:
                    o = xin[:, 0 if s == "s" else 1, j0 : j0 + ln // 128, :, :]
                    o = o.rearrange("p j h f -> p j (h f)")
                nc.gpsimd.dma_gather(
                    out_ap=o,
                    in_ap=tab[w0:w1, :],
                    idxs_ap=ti[s][:, (col0 - base) // 16 : (col0 - base + ln) // 16],
                    num_idxs=ln,
                    num_idxs_reg=ln,
                    elem_size=128,
                    queue_num=q,
                    single_packet=False,
                )

            # in-place pair half-selects (bot -> top where parity=1).
            # NOTE: splitting these per slot-half regresses 7.7% — gather
            # segments span the half boundary, so nothing decouples.
            mb = {
                s: mt[:, i, :].unsqueeze(2).broadcast_to((128, J, 64))
                for i, s in enumerate("sdv")
            }
            # src_sel in place, then pack dst_sel into src's bot half so the
            # transpose input xin[:, 0, j, :, :] is a contiguous [128, 128]
            nc.vector.copy_predicated(
                out=xin[:, 0, :, 0, :], mask=mb["s"], data=xin[:, 0, :, 1, :]
            )
            nc.vector.select(
                out=xin[:, 0, :, 1, :], mask=mb["d"],
                on_true=xin[:, 1, :, 1, :], on_false=xin[:, 1, :, 0, :],
            )
            nc.vector.copy_predicated(
                out=P_v[:, :, 0:64], mask=mb["v"], data=P_v[:, :, 64:128]
            )

            # xt tiles (feature-major rhs chunks); xt2 = [afT(0:64); vtx(64:128)]
            xt1 = x_pool.tile([128, SB], BF16, tag="xt1")
            xt2 = x_pool.tile([128, SB], BF16, tag="xt2")
            nc.sync.dma_start(
                out=xt2[0:64, :], in_=afT[:, base : base + SB]
            )

            # transposes: 4 slots per PSUM tile
            for g4 in range(J // 4):
                tpt = tp_psum.tile([128, 1024], BF16, tag="tp")
                c1p = tpt[:, 0:512]
                vtp = tpt[64:128, 512:1024]
                for u in range(4):
                    j = g4 * 4 + u
                    nc.tensor.transpose(
                        out=c1p[:, u * 128 : (u + 1) * 128],
                        in_=xin[:, 0, j, :, :],
                        identity=ident[:],
                    )
                    nc.tensor.transpose(
                        out=vtp[:, u * 128 : (u + 1) * 128],
                        in_=P_v[:, j, 0:64],
                        identity=ident[:],
                    )
                sl = slice(g4 * 512, (g4 + 1) * 512)
                # 2-of-8 on DVE / 6 on ACT is the measured sweet spot: all-ACT
                # makes the scalar chain the pace-setter (+28%), 4/4 is ~3%
                # slower than this.
                if g4 % 4 == 0:
                    nc.vector.tensor_copy(out=xt1[:, sl], in_=c1p[:])
                else:
                    nc.scalar.copy(out=xt1[:, sl], in_=c1p[:])
                nc.scalar.copy(out=xt2[64:128, sl], in_=vtp[:])

            # epilogue accumulators (feature-major, whole superblock);
            # silu overwrites soS in place, result lands in gsS
            gsS = ep_pool.tile([64, SB], BF16, tag="gsS")
            soS = ep_pool.tile([64, SB], BF16, tag="soS")

            # MLPs per 512-col subtile
            for g4 in range(SB // SUB):
                sl = slice(g4 * SUB, (g4 + 1) * SUB)
                hgp = h_psum.tile([128, SUB], F32, tag="hg")
                hop = h_psum.tile([128, SUB], F32, tag="ho")
                nc.tensor.matmul(
                    out=hgp[:], lhsT=w1b[:, 0, 0, :], rhs=xt1[:, sl],
                    start=True, stop=False,
                )
                nc.tensor.matmul(
                    out=hgp[:], lhsT=w1b[:, 1, 0, :], rhs=xt2[:, sl],
                    start=False, stop=True,
                )
                nc.tensor.matmul(
                    out=hop[:], lhsT=w1b[:, 0, 1, :], rhs=xt1[:, sl],
                    start=True, stop=False,
                )
                nc.tensor.matmul(
                    out=hop[:], lhsT=w1b[:, 1, 1, :], rhs=xt2[:, sl],
                    start=False, stop=True,
                )

                hg = h_pool.tile([128, SUB], BF16, tag="hg")
                nc.scalar.activation(out=hg[:], in_=hgp[:], func=Relu, bias=gb1t[:, 0:1])
                ho = h_pool.tile([128, SUB], BF16, tag="ho")
                if g4 % 2 == 0:
                    nc.vector.tensor_scalar(
                        out=ho[:], in0=hop[:],
                        scalar1=ob1t[:, 0:1], scalar2=0.0,
                        op0=mybir.AluOpType.add, op1=mybir.AluOpType.max,
                    )
                else:
                    nc.scalar.activation(
                        out=ho[:], in_=hop[:], func=Relu, bias=ob1t[:, 0:1]
                    )

                m2 = m2_psum.tile([128, SUB], F32, tag="m2")
                nc.tensor.matmul(
                    out=m2[0:64, :], lhsT=gw2t[:], rhs=hg[:], start=True, stop=True
                )
                nc.tensor.matmul(
                    out=m2[64:128, :], lhsT=ow2t[:], rhs=ho[:], start=True, stop=True
                )

                # NOTE: computing silu via func=Silu here regresses 65% —
                # alternating Sigmoid/Silu forces ACT LUT-set reloads.
                nc.scalar.activation(
                    out=gsS[:, sl], in_=m2[0:64, :], func=Sigmoid, bias=gb2t[:, 0:1]
                )
                nc.scalar.activation(
                    out=soS[:, sl], in_=m2[64:128, :], func=Sigmoid, bias=ob2t[:, 0:1]
                )
                # silu in place: soS <- (m2o + b2) * sigmoid(m2o + b2)
                nc.vector.scalar_tensor_tensor(
                    out=soS[:, sl], in0=m2[64:128, :], scalar=ob2t[:, 0:1],
                    in1=soS[:, sl],
                    op0=mybir.AluOpType.add, op1=mybir.AluOpType.mult,
                )

            if debug_taps and sb_i == 0:
                nc.sync.dma_start(out=dbg["dbg_xin"], in_=xin[:].rearrange("p s j h f -> p (s j h f)"))
                nc.sync.dma_start(out=dbg["dbg_pv"], in_=P_v[:].rearrange("p j f -> p (j f)"))
                nc.sync.dma_start(out=dbg["dbg_xt1"], in_=xt1[:])
                nc.sync.dma_start(out=dbg["dbg_xt2"], in_=xt2[:])
                nc.sync.dma_start(out=dbg["dbg_gs"], in_=gsS[:])
                nc.sync.dma_start(out=dbg["dbg_ob"], in_=soS[:])

            # batched epilogue: res = af + sigmoid(g) * silu(o); halved so the
            # first half's outT store overlaps the second half's math
            for h in range(2):
                hl = slice(h * (SB // 2), (h + 1) * (SB // 2))
                nc.vector.tensor_mul(
                    out=gsS[:, hl], in0=gsS[:, hl], in1=soS[:, hl]
                )
                nc.vector.tensor_add(
                    out=gsS[:, hl], in0=gsS[:, hl], in1=xt2[0:64, hl]
                )
                nc.sync.dma_start(
                    out=outT[:, base + h * (SB // 2) : base + (h + 1) * (SB // 2)],
                    in_=gsS[:, hl],
                )

    nc.compile()
    return nc


# ---------------------------------------------------------------- entry points

_CACHE = {}


def run(inputs, trace=False, **kw):
    from concourse.bass_utils import run_bass_kernel_spmd
    import ml_dtypes

    src = np.asarray(inputs["edge_src"]).astype(np.int64)
    dst = np.asarray(inputs["edge_dst"]).astype(np.int64)
    vtx = np.asarray(inputs["angle_index"])[:, 1].astype(np.int64)

    lay = build_layout(src, dst, vtx)
    A = lay["A"]

    sig = (A, tuple(lay["cap"].tolist()))
    if sig not in _CACHE:
        _CACHE.clear()
        _CACHE[sig] = build_nc(A, lay["segs"])
    nc = _CACHE[sig]

    bondP = pair_pack(np.asarray(inputs["bond_feat"], np.float32))
    atomP = pair_pack(np.asarray(inputs["atom_feat"], np.float32))
    gW1 = np.asarray(inputs["gW1"], np.float32)
    oW1 = np.asarray(inputs["oW1"], np.float32)
    # x chunks: xt1 = [src(0:64); dst(64:128)], xt2 = [vtx(0:64); af(64:128)]
    # original x = [src, dst, af, vtx]
    # xt1 = [src(0:64); dst(64:128)], xt2 = [af(0:64); vtx(64:128)]
    w1c1 = np.stack([gW1[0:128], oW1[0:128]], axis=1)  # [128, 2, HID]
    w1c2 = np.stack([gW1[128:256], oW1[128:256]], axis=1)

    common = {
        "bondP": bondP,
        "atomP": atomP,
        "w1c1": np.ascontiguousarray(w1c1),
        "w1c2": np.ascontiguousarray(w1c2),
        "gb1": np.asarray(inputs["gb1"], np.float32),
        "ob1": np.asarray(inputs["ob1"], np.float32),
        "gW2": np.asarray(inputs["gW2"], np.float32),
        "oW2": np.asarray(inputs["oW2"], np.float32),
        "gb2": np.asarray(inputs["gb2"], np.float32),
        "ob2": np.asarray(inputs["ob2"], np.float32),
        "ident128": np.eye(128, dtype=np.float32).astype(ml_dtypes.bfloat16),
    }

    af = np.asarray(inputs["angle_feat"], np.float32)  # [N_ANGLES, D]
    in_maps = []
    for c in range(N_CORES):
        p = lay["pack"][c]
        afT = np.zeros((D, A), ml_dtypes.bfloat16)
        valid = p["angle_of_col"] >= 0
        afT[:, valid] = af[p["angle_of_col"][valid]].T.astype(ml_dtypes.bfloat16)
        m = dict(common)
        m["afT"] = afT
        m["idx_s"], m["idx_d"], m["idx_v"] = p["idx_s"], p["idx_d"], p["idx_v"]
        m["masks"] = p["masks"]
        in_maps.append(m)

    r = run_bass_kernel_spmd(nc, in_maps, core_ids=list(range(N_CORES)), trace=trace, **kw)

    out = np.empty((N_ANGLES, D), np.float32)
    for c in range(N_CORES):
        p = lay["pack"][c]
        oT = r.results[c]["outT"].astype(np.float32)  # [64, A]
        valid = p["angle_of_col"] >= 0
        out[p["angle_of_col"][valid]] = oT[:, valid].T
    return out, r


def kernel(**inputs):
    out, _ = run(inputs, trace=False)
    return out

